# revision 12
# baseline (speedup 1.0000x reference)
"""7x7 valid conv2d (cross-correlation) on a 4096x4096 fp32 image, 8 NeuronCores.

Strategy: 2x4 core grid (2 row bands x 4 col bands), halo baked in on the host.
Per core the conv runs on the TensorEngine as a 4x4 array-packed (tile_position)
banded-Toeplitz matmul: the 128x128 PE array is split into 16 independent 32x32
sub-arrays.  Sub-array (i,j) contracts a 32-row input strip (SBUF partitions
[32i,32i+32) of moving stream j) against a [32,26] Toeplitz band of kernel
column kx, producing 26 output rows into PSUM bank i partitions [32j,32j+26).
Streams j=0..3 each carry 4 input strips (26-row pitch packed at 32-partition
pitch), so one "round" of 16 concurrent tile-matmuls covers 416 output rows
for one kx; 7 rounds accumulate the full conv.  This streams ~10x more moving
columns per cycle than a full-array matmul whose stationary band is 95% zeros.

PSUM has_written semantics: start=True clears flags for the WHOLE bank, so per
(block, bank) exactly one matmul (the first) carries start=True; later writes
to untouched elements overwrite-and-mark, accumulating thereafter.
"""

import numpy as np
import ml_dtypes

import concourse.bacc as bacc
import concourse.bass as bass
import concourse.tile as tile
import concourse.mybir as mybir
from concourse.bass_utils import run_bass_kernel_spmd

H = W = 4096
KH = KW = 7
OH = OW = H - KH + 1  # 4090
NCORES = 8
GR, GC = 2, 4
ROWS_PC = 2045
COLS_PC = 1023
ROW_BAND = [0, 2045]
COL_BAND = [0, 1023, 2046, 3067]

SG = 26                       # output rows per 32-row strip
RS_ROWS = 16 * SG             # 416 output rows per round-set
RS_STARTS = [0, 416, 832, 1248, 1664]
IN_ROWS = RS_STARTS[-1] + 416 + 8   # 2088 (2051 real + pad)
IN_COLS = 1032
COL_TILES = [(0, 512), (512, 511)]
NRS = len(RS_STARTS)

MODE = "bf16"
TRACE = False
LAST_EXEC_NS = None

_DT = {"bf16": (mybir.dt.bfloat16, ml_dtypes.bfloat16)}
_compiled = {}


def _build(mode):
    dt_b, _ = _DT[mode]
    nc = bacc.Bacc(
        "TRN2", target_bir_lowering=False, debug=False, num_devices=NCORES
    )
    x_d = nc.dram_tensor("x", [IN_ROWS, IN_COLS], dt_b, kind="ExternalInput").ap()
    t_d = nc.dram_tensor("tmats", [128, KW * 32], dt_b, kind="ExternalInput").ap()
    # out[rs, ct, p, 512*i + c] = conv row rs*416 + 104*(p//32) + 26*i + (p%32),
    # col 512*ct + c   (p%32 >= 26 rows are garbage, host drops them)
    o_d = nc.dram_tensor(
        "out", [NRS, 2, 128, 2048], mybir.dt.bfloat16, kind="ExternalOutput"
    ).ap()

    with tile.TileContext(nc) as tc:
        with (
            tc.tile_pool(name="tmat", bufs=1) as tpool,
            tc.tile_pool(name="xsl", bufs=2) as xpool,
            tc.tile_pool(name="ps", bufs=2, space="PSUM") as ppool,
            tc.tile_pool(name="ost", bufs=4) as opool,
        ):
            ts = tpool.tile([128, KW * 32], dt_b)
            nc.sync.dma_start(ts[:], t_d[:])
            for rs, r0 in enumerate(RS_STARTS):
                # 4 moving streams; stream j partitions [32i,32i+32) carry
                # x rows [r0 + 104j + 26i, +32)
                mts = []
                for j in range(4):
                    mj = xpool.tile([128, IN_COLS], dt_b, tag=f"m{j}", name=f"m{j}")
                    for i in range(4):
                        rr = r0 + 104 * j + SG * i
                        nc.sync.dma_start(
                            mj[32 * i : 32 * i + 32, :], x_d[rr : rr + 32, :]
                        )
                    mts.append(mj)
                for ct, (c0, N) in enumerate(COL_TILES):
                    pb = [
                        ppool.tile(
                            [128, 512], mybir.dt.float32, tag=f"pb{i}", name=f"pb{i}"
                        )
                        for i in range(4)
                    ]
                    for r in range(KW):
                        for j in range(4):
                            for i in range(4):
                                # exactly one start=True per bank (first write)
                                nc.tensor.matmul(
                                    pb[i][32 * j : 32 * j + SG, :N],
                                    ts[32 * i : 32 * i + 32, 32 * r : 32 * r + SG],
                                    mts[j][32 * i : 32 * i + 32, c0 + r : c0 + r + N],
                                    start=(r == 0),
                                    stop=(r == KW - 1),
                                    tile_position=(32 * i, 32 * j),
                                    skip_group_check=True,
                                )
                    ot = opool.tile([128, 2048], mybir.dt.bfloat16, tag="o")
                    for i in range(4):
                        eng = nc.vector if i < 2 else nc.scalar
                        if i < 2:
                            eng.tensor_copy(
                                ot[:, 512 * i : 512 * i + N], pb[i][:, :N]
                            )
                        else:
                            eng.copy(ot[:, 512 * i : 512 * i + N], pb[i][:, :N])
                    nc.gpsimd.dma_start(o_d[rs, ct, :, :], ot[:, :])
    nc.compile()
    return nc


def _toeplitz(weight, np_dt):
    t = np.zeros((128, KW * 32), dtype=np.float32)
    for i in range(4):
        for kx in range(KW):
            for ky in range(KH):
                for m in range(SG):
                    t[32 * i + m + ky, kx * 32 + m] = weight[ky, kx]
    return np.ascontiguousarray(t.astype(np_dt))


def kernel(x, weight):
    global LAST_EXEC_NS
    mode = MODE
    dt_b, np_dt = _DT[mode]
    if mode not in _compiled:
        _compiled[mode] = _build(mode)
    nc = _compiled[mode]

    xf = np.asarray(x, np.float32)
    wf = np.asarray(weight, np.float32)
    tmats = _toeplitz(wf, np_dt)
    xc = xf.astype(np_dt)

    xpad = np.zeros((ROW_BAND[-1] + IN_ROWS, COL_BAND[-1] + IN_COLS), dtype=xc.dtype)
    xpad[:H, :W] = xc
    in_maps = []
    for c in range(NCORES):
        r0, c0 = ROW_BAND[c // GC], COL_BAND[c % GC]
        in_maps.append(
            {
                "x": np.ascontiguousarray(xpad[r0 : r0 + IN_ROWS, c0 : c0 + IN_COLS]),
                "tmats": tmats,
            }
        )
    res = run_bass_kernel_spmd(
        nc, in_maps, core_ids=list(range(NCORES)), trace=TRACE
    )
    LAST_EXEC_NS = res.exec_time_ns

    out = np.empty((OH, OW), np.float32)
    for c in range(NCORES):
        rb, cb = ROW_BAND[c // GC], COL_BAND[c % GC]
        od = res.results[c]["out"]  # [NRS, 2, 128, 2048] bf16
        core = np.empty((RS_STARTS[-1] + 416, COLS_PC), np.float32)
        for rs, r0 in enumerate(RS_STARTS):
            for ct, (c0, N) in enumerate(COL_TILES):
                blk = od[rs, ct].astype(np.float32)  # [128, 2048]
                for j in range(4):
                    for i in range(4):
                        rows = r0 + 104 * j + SG * i
                        core[rows : rows + SG, c0 : c0 + N] = blk[
                            32 * j : 32 * j + SG, 512 * i : 512 * i + N
                        ]
        out[rb : rb + ROWS_PC, cb : cb + COLS_PC] = core[:ROWS_PC]
    return out


# revision 15
# speedup vs baseline: 1.0730x; 1.0730x over previous
"""7x7 valid conv2d (cross-correlation) on a 4096x4096 fp32 image, 8 NeuronCores.

Strategy: 2x4 core grid (2 row bands x 4 col bands), halo baked in on the host
so there are no device collectives.  Per core the conv runs on the TensorEngine
as 7 PSUM-accumulated "banded Toeplitz" matmuls per (row-stripe, col-tile):
for each kernel column kx, a [K=128, M=122] stationary matrix T_kx with
T_kx[m+ky, m] = w[ky, kx] contracts 128 input rows into 122 output rows; the
kx shift is a free column offset on the moving operand.

vs the old 1x8 row split this cuts matmuls/core from 280 to 238: each core
runs 17 stripes x 2 col-tiles x 7 kx instead of 5 stripes x 8 col-tiles x 7
(the 8-way row split wasted 16% of PE cycles on a 24-rows-kept 5th stripe).
"""

import numpy as np
import ml_dtypes

import concourse.bacc as bacc
import concourse.bass as bass
import concourse.tile as tile
import concourse.mybir as mybir
from concourse.bass_utils import run_bass_kernel_spmd

H = W = 4096
KH = KW = 7
OH = OW = H - KH + 1  # 4090
NCORES = 8
GR, GC = 2, 4                  # core grid: 2 row bands x 4 col bands
ROWS_PC = 2045                 # output rows per core
COLS_PC = 1023                 # output cols per core (col bands overlap by 2)
ROW_BAND = [0, 2045]
COL_BAND = [0, 1023, 2046, 3067]   # last band overlaps band 2 by 2 cols
MT = 122                       # output rows per stripe (contraction K = 128)
ROW_STARTS = list(range(0, ROWS_PC, MT))  # 17 stripes, last keeps 93 rows
IN_ROWS = ROW_STARTS[-1] + 128            # 2080 (2051 real + pad)
IN_COLS = 1032                            # 1023 + 6 halo + 3 pad
OUT_COLS = 1040                           # 1023 + pad: non-contig DRAM rows keep
                                          # store descriptors sprayed across queues
COL_TILES = [(0, 512), (512, 511)]        # (c0, N) psum col tiles

MODE = "bf16"
TRACE = False
LAST_EXEC_NS = None

_DT = {
    "bf16": (mybir.dt.bfloat16, ml_dtypes.bfloat16),
    "fp32": (mybir.dt.float32, np.float32),
}

_compiled = {}


def _build(mode):
    dt_b, _ = _DT[mode]
    nc = bacc.Bacc(
        "TRN2", target_bir_lowering=False, debug=False, num_devices=NCORES
    )
    x_d = nc.dram_tensor("x", [IN_ROWS, IN_COLS], dt_b, kind="ExternalInput").ap()
    t_d = nc.dram_tensor("tmats", [128, KW * MT], dt_b, kind="ExternalInput").ap()
    # bf16 output staging/store halves store traffic; host upcasts.
    o_d = nc.dram_tensor(
        "out", [ROWS_PC, OUT_COLS], mybir.dt.bfloat16, kind="ExternalOutput"
    ).ap()

    with tile.TileContext(nc) as tc:
        with (
            tc.tile_pool(name="tmat", bufs=1) as tpool,
            tc.tile_pool(name="xsl", bufs=6) as xpool,
            tc.tile_pool(name="ps", bufs=8, space="PSUM") as ppool,
            tc.tile_pool(name="ost", bufs=8) as opool,
        ):
            tm = tpool.tile([128, KW * MT], dt_b)
            nc.sync.dma_start(tm[:], t_d[:])
            for ti, r0 in enumerate(ROW_STARTS):
                M = min(MT, ROWS_PC - r0)  # rows actually kept
                xt = xpool.tile([128, IN_COLS], dt_b, tag="x")
                if ti == 0:
                    # first matmul group only needs cols 0-518: land it early
                    nc.sync.dma_start(xt[:, :524], x_d[r0 : r0 + 128, :524])
                    nc.sync.dma_start(xt[:, 524:], x_d[r0 : r0 + 128, 524:])
                else:
                    nc.sync.dma_start(xt[:, :], x_d[r0 : r0 + 128, :])
                ot = opool.tile([128, COLS_PC], mybir.dt.bfloat16, tag="o")
                for ci, (c0, N) in enumerate(COL_TILES):
                    ps = ppool.tile([MT, 512], mybir.dt.float32, tag="ps")
                    for kx in range(KW):
                        nc.tensor.matmul(
                            ps[:, :N],
                            tm[:, kx * MT : kx * MT + MT],
                            xt[:, c0 + kx : c0 + kx + N],
                            start=(kx == 0),
                            stop=(kx == KW - 1),
                        )
                    # PSUM drain split across DVE and ACT so the two col-tiles'
                    # casts run in parallel (shortens the end-of-stream drain)
                    if ci == 0:
                        nc.vector.tensor_copy(ot[:M, c0 : c0 + N], ps[:M, :N])
                    else:
                        nc.scalar.copy(ot[:M, c0 : c0 + N], ps[:M, :N])
                    # SWDGE (gpsimd) store: sprays descriptors across all 16
                    # SDMA engines. HWDGE funnels this pattern onto 2 queues
                    # (~54 GB/s) - measured, do not switch.  Last stripes ship
                    # per col-tile so the final drain chain is short.
                    if ti >= len(ROW_STARTS) - 3:
                        nc.gpsimd.dma_start(
                            o_d[r0 : r0 + M, c0 : c0 + N], ot[:M, c0 : c0 + N]
                        )
                if ti < len(ROW_STARTS) - 3:
                    nc.gpsimd.dma_start(o_d[r0 : r0 + M, :COLS_PC], ot[:M, :COLS_PC])
    nc.compile()
    return nc


def _toeplitz(weight, np_dt):
    t = np.zeros((128, KW * MT), dtype=np.float32)
    idx = np.arange(MT)
    for kx in range(KW):
        for ky in range(KH):
            t[idx + ky, kx * MT + idx] = weight[ky, kx]
    return np.ascontiguousarray(t.astype(np_dt))


def kernel(x, weight):
    global LAST_EXEC_NS
    mode = MODE
    dt_b, np_dt = _DT[mode]
    if mode not in _compiled:
        _compiled[mode] = _build(mode)
    nc = _compiled[mode]

    xf = np.asarray(x, np.float32)
    wf = np.asarray(weight, np.float32)
    tmats = _toeplitz(wf, np_dt)
    xc = xf.astype(np_dt) if np_dt is not np.float32 else xf

    # padded canvas so every core's slab is [IN_ROWS, IN_COLS]
    xpad = np.zeros((ROW_BAND[-1] + IN_ROWS, COL_BAND[-1] + IN_COLS), dtype=xc.dtype)
    xpad[:H, :W] = xc
    in_maps = []
    for c in range(NCORES):
        r0, c0 = ROW_BAND[c // GC], COL_BAND[c % GC]
        in_maps.append(
            {
                "x": np.ascontiguousarray(xpad[r0 : r0 + IN_ROWS, c0 : c0 + IN_COLS]),
                "tmats": tmats,
            }
        )
    res = run_bass_kernel_spmd(
        nc, in_maps, core_ids=list(range(NCORES)), trace=TRACE
    )
    LAST_EXEC_NS = res.exec_time_ns

    out = np.empty((OH, OW), np.float32)
    for c in range(NCORES):
        r0, c0 = ROW_BAND[c // GC], COL_BAND[c % GC]
        out[r0 : r0 + ROWS_PC, c0 : c0 + COLS_PC] = res.results[c]["out"][
            :, :COLS_PC
        ].astype(np.float32)
    return out
